# revision 12
# baseline (speedup 1.0000x reference)
"""Associative-memory KNN retrieval kernel for 8 TRN2 NeuronCores.

Strategy: data-parallel shard of queries (1024/core), keys/values replicated.
Per core:
  - normalize queries (x |temp|), transpose to [D, N] bf16
  - stream keys in m-chunks of 1024: normalize, transpose via PE, matmul
    scores [128q, 1024m] into PSUM (bf16 operands, f32 accum)
  - pack scores: (score & 0xFFFF8000) | key_idx  -> top-8 per chunk via
    DVE max8 -> 256 packed candidates per query row
  - stage B: 4 rounds of max8 + match_replace -> sorted top-32 (packed)
  - unpack indices, softmax on quantized scores, indirect-DMA gather of
    value rows, weighted sum, residual + layernorm
"""

import sys

import numpy as np

import concourse.bass as bass
import concourse.bacc as bacc_mod
import concourse.mybir as mybir
from concourse.bass_utils import run_bass_kernel_spmd
from concourse.masks import make_identity
from concourse.tile import TileContext

P = 128
D = 1024
M = 32768
TOPK = 32
N_CORES = 8
N_PER_CORE = 1024
NB = N_PER_CORE // P          # 8 query blocks per core
CH = 1024                     # m-chunk size
NCH = M // CH                 # 32 chunks
DT = D // P                   # 8 d-tiles
MT = CH // P                  # 8 m-tiles per chunk
SEG8 = 8                      # max8 returns 8 candidates per chunk
NCAND = NCH * SEG8            # 256 candidates per row

MASK_HI = 0xFFFF8000          # keep sign+exp+9 mantissa bits
MASK_LO = 0x00007FFF          # 15-bit index (M = 2^15)
NEG_BIG = -3.0e38

F32 = mybir.dt.float32
BF16 = mybir.dt.bfloat16
U32 = mybir.dt.uint32
I32 = mybir.dt.int32

LN_EPS = 1e-5
NORM_EPS = 1e-12

_NC_CACHE = {}
LAST_RESULT = None


def build_nc():
    nc = bacc_mod.Bacc()

    q_ext = nc.declare_dram_parameter("query", [N_PER_CORE, D], F32, isOutput=False)
    k_ext = nc.declare_dram_parameter("memory_keys", [M, D], F32, isOutput=False)
    v_ext = nc.declare_dram_parameter("memory_values", [M, D], F32, isOutput=False)
    t_ext = nc.declare_dram_parameter("temperature", [1, 1], F32, isOutput=False)
    g_ext = nc.declare_dram_parameter("ln_gamma", [1, D], F32, isOutput=False)
    b_ext = nc.declare_dram_parameter("ln_beta", [1, D], F32, isOutput=False)
    out_ext = nc.declare_dram_parameter("out", [N_PER_CORE, D], F32, isOutput=True)
    attn_ext = nc.declare_dram_parameter("attn", [N_PER_CORE, TOPK], F32, isOutput=True)

    with TileContext(nc) as tc:
        with (
            tc.tile_pool(name="const", bufs=1) as const_pool,
            tc.tile_pool(name="persist", bufs=1) as persist_pool,
            tc.tile_pool(name="kraw", bufs=3) as kraw_pool,
            tc.tile_pool(name="kn16", bufs=MT + 2) as kn16_pool,
            tc.tile_pool(name="knt", bufs=2) as knt_pool,
            tc.tile_pool(name="iota", bufs=2) as iota_pool,
            tc.tile_pool(name="packed", bufs=3) as packed_pool,
            tc.tile_pool(name="small", bufs=4) as small_pool,
            tc.tile_pool(name="junk", bufs=2) as junk_pool,
            tc.tile_pool(name="vg", bufs=3) as vg_pool,
            tc.tile_pool(name="wtmp", bufs=2) as wtmp_pool,
            tc.tile_pool(name="big2", bufs=2) as big2_pool,
            tc.tile_pool(name="psum_s", bufs=2, space="PSUM") as psum_s_pool,
            tc.tile_pool(name="psum_t", bufs=2, space="PSUM") as psum_t_pool,
        ):
            # ---------------- phase 0: constants + queries ----------------
            identity = const_pool.tile([P, P], BF16)
            make_identity(nc, identity[:])

            tt = const_pool.tile([P, 1], F32)
            nc.sync.dma_start(out=tt[:], in_=t_ext[0:1, 0:1].to_broadcast([P, 1]))
            tt_abs = const_pool.tile([P, 1], F32)
            nc.scalar.activation(out=tt_abs[:], in_=tt[:], func=mybir.ActivationFunctionType.Abs)

            eps_t = const_pool.tile([P, 1], F32)
            nc.vector.memset(eps_t[:], LN_EPS)
            mask_hi_t = const_pool.tile([P, 1], U32)
            nc.vector.memset(mask_hi_t[:], MASK_HI)
            mask_lo_t = const_pool.tile([P, 1], U32)
            nc.vector.memset(mask_lo_t[:], MASK_LO)

            gamma_bc = const_pool.tile([P, D], F32)
            nc.sync.dma_start(out=gamma_bc[:], in_=g_ext[0:1, :].to_broadcast([P, D]))
            beta_bc = const_pool.tile([P, D], F32)
            nc.sync.dma_start(out=beta_bc[:], in_=b_ext[0:1, :].to_broadcast([P, D]))

            # qnT layout: [P(d within tile), DT(d tile), N_PER_CORE(q)]
            qnT = persist_pool.tile([P, DT, N_PER_CORE], BF16)
            for b in range(NB):
                qblk = kraw_pool.tile([P, D], F32, tag="qblk", bufs=2)
                nc.sync.dma_start(out=qblk[:], in_=q_ext[b * P:(b + 1) * P, :])
                junk = junk_pool.tile([P, D], BF16)
                ssq = small_pool.tile([P, 1], F32, tag="ssq_q")
                nc.scalar.activation(
                    out=junk[:], in_=qblk[:],
                    func=mybir.ActivationFunctionType.Square, accum_out=ssq[:],
                )
                qs = small_pool.tile([P, 1], F32, tag="qs")
                nc.scalar.activation(out=qs[:], in_=ssq[:], func=mybir.ActivationFunctionType.Sqrt)
                nc.vector.tensor_scalar_max(qs[:], qs[:], NORM_EPS)
                rq = small_pool.tile([P, 1], F32, tag="rq")
                nc.vector.reciprocal(out=rq[:], in_=qs[:])
                qscale = small_pool.tile([P, 1], F32, tag="qscale")
                nc.vector.tensor_tensor(out=qscale[:], in0=rq[:], in1=tt_abs[:], op=mybir.AluOpType.mult)
                qn16 = kn16_pool.tile([P, D], BF16, tag="qn16", bufs=2)
                nc.vector.tensor_scalar(
                    out=qn16[:], in0=qblk[:], scalar1=qscale[:, 0:1], scalar2=None,
                    op0=mybir.AluOpType.mult,
                )
                for dt in range(DT):
                    pq = psum_t_pool.tile([P, P], BF16, tag="pq")
                    nc.tensor.transpose(pq[:], qn16[:, dt * P:(dt + 1) * P], identity[:])
                    nc.vector.tensor_copy(out=qnT[:, dt, b * P:(b + 1) * P], in_=pq[:])

            # candidate buffers per block: packed f32 [P, NCAND]
            cands = [
                persist_pool.tile([P, NCAND], F32, tag=f"cand{b}", name=f"cand{b}")
                for b in range(NB)
            ]

            # ---------------- phase 1: stream m-chunks ----------------
            for c in range(NCH):
                kn16s = []
                for t in range(MT):
                    kr = kraw_pool.tile([P, D], F32)
                    nc.sync.dma_start(
                        out=kr[:], in_=k_ext[c * CH + t * P: c * CH + (t + 1) * P, :]
                    )
                    junkk = junk_pool.tile([P, D], BF16)
                    ssqk = small_pool.tile([P, 1], F32, tag="ssqk")
                    nc.scalar.activation(
                        out=junkk[:], in_=kr[:],
                        func=mybir.ActivationFunctionType.Square,
                        accum_out=ssqk[:],
                    )
                    sdk = small_pool.tile([P, 1], F32, tag="sdk")
                    nc.scalar.activation(out=sdk[:], in_=ssqk[:], func=mybir.ActivationFunctionType.Sqrt)
                    nc.vector.tensor_scalar_max(sdk[:], sdk[:], NORM_EPS)
                    rk = small_pool.tile([P, 1], F32, tag="rk")
                    nc.vector.reciprocal(out=rk[:], in_=sdk[:])
                    kn = kn16_pool.tile([P, D], BF16)
                    nc.vector.tensor_scalar(
                        out=kn[:], in0=kr[:], scalar1=rk[:, 0:1], scalar2=None,
                        op0=mybir.AluOpType.mult,
                    )
                    kn16s.append(kn)

                # knT chunk: [P(d within), DT(d tile), CH(m)]
                knt = knt_pool.tile([P, DT, CH], BF16)
                for dt in range(DT):
                    for g in range(2):
                        pt = psum_t_pool.tile([P, 512], BF16, tag="pt")
                        for j in range(4):
                            t = g * 4 + j
                            nc.tensor.transpose(
                                pt[:, j * P:(j + 1) * P],
                                kn16s[t][:, dt * P:(dt + 1) * P],
                                identity[:],
                            )
                        nc.vector.tensor_copy(out=knt[:, dt, g * 512:(g + 1) * 512], in_=pt[:])

                iota_t = iota_pool.tile([P, CH], U32)
                nc.gpsimd.iota(
                    out=iota_t[:], pattern=[[1, CH]], base=c * CH, channel_multiplier=0
                )

                for b in range(NB):
                    ps = psum_s_pool.tile([P, CH], F32)
                    for d in range(DT):
                        for half in range(2):
                            nc.tensor.matmul(
                                out=ps[:, half * 512:(half + 1) * 512],
                                lhsT=qnT[:, d, b * P:(b + 1) * P],
                                rhs=knt[:, d, half * 512:(half + 1) * 512],
                                start=(d == 0),
                                stop=(d == DT - 1),
                            )
                    packed = packed_pool.tile([P, CH], U32)
                    nc.vector.tensor_scalar(
                        out=packed[:], in0=ps[:].bitcast(U32), scalar1=mask_hi_t[:, 0:1],
                        scalar2=None, op0=mybir.AluOpType.bitwise_and,
                    )
                    # low 15 bits are zeroed by the mask, so integer add == bitwise or
                    # (32-bit bitwise ops are DVE-only; Pool supports integer add)
                    nc.gpsimd.tensor_tensor(
                        out=packed[:], in0=packed[:], in1=iota_t[:],
                        op=mybir.AluOpType.add,
                    )
                    nc.vector.max(
                        out=cands[b][:, c * SEG8:(c + 1) * SEG8],
                        in_=packed[:].bitcast(F32),
                    )

            # ---------------- phase 2: per-block top-k + gather ----------------
            for b in range(NB):
                cand = cands[b]
                win = small_pool.tile([P, TOPK], F32, tag="win")
                for r in range(TOPK // 8):
                    nc.vector.max(out=win[:, r * 8:(r + 1) * 8], in_=cand[:])
                    nc.vector.match_replace(
                        out=cand[:], in_to_replace=win[:, r * 8:(r + 1) * 8],
                        in_values=cand[:], imm_value=NEG_BIG,
                    )
                idx32 = small_pool.tile([P, TOPK], U32, tag="idx32")
                nc.vector.tensor_scalar(
                    out=idx32[:], in0=win[:].bitcast(U32), scalar1=mask_lo_t[:, 0:1],
                    scalar2=None, op0=mybir.AluOpType.bitwise_and,
                )
                idx_i = small_pool.tile([P, TOPK], I32, tag="idx_i")
                nc.vector.tensor_copy(out=idx_i[:], in_=idx32[:])
                scq = small_pool.tile([P, TOPK], U32, tag="scq")
                nc.vector.tensor_scalar(
                    out=scq[:], in0=win[:].bitcast(U32), scalar1=mask_hi_t[:, 0:1],
                    scalar2=None, op0=mybir.AluOpType.bitwise_and,
                )
                scqf = scq[:].bitcast(F32)
                negmx = small_pool.tile([P, 1], F32, tag="negmx")
                nc.vector.tensor_scalar(
                    out=negmx[:], in0=scqf[:, 0:1], scalar1=-1.0, scalar2=None,
                    op0=mybir.AluOpType.mult,
                )
                ex = small_pool.tile([P, TOPK], F32, tag="ex")
                nc.scalar.activation(
                    out=ex[:], in_=scqf, func=mybir.ActivationFunctionType.Exp,
                    bias=negmx[:, 0:1], scale=1.0,
                )
                ssum = small_pool.tile([P, 1], F32, tag="ssum")
                nc.vector.tensor_reduce(
                    out=ssum[:], in_=ex[:], axis=mybir.AxisListType.XYZW,
                    op=mybir.AluOpType.add,
                )
                rs = small_pool.tile([P, 1], F32, tag="rs")
                nc.vector.reciprocal(out=rs[:], in_=ssum[:])
                attn_t = small_pool.tile([P, TOPK], F32, tag="attn_t")
                nc.vector.tensor_scalar(
                    out=attn_t[:], in0=ex[:], scalar1=rs[:, 0:1], scalar2=None,
                    op0=mybir.AluOpType.mult,
                )
                nc.sync.dma_start(out=attn_ext[b * P:(b + 1) * P, :], in_=attn_t[:])

                acc = big2_pool.tile([P, D], F32, tag="acc")
                for k in range(TOPK):
                    vg = vg_pool.tile([P, D], F32)
                    nc.gpsimd.indirect_dma_start(
                        out=vg[:], out_offset=None,
                        in_=v_ext[:],
                        in_offset=bass.IndirectOffsetOnAxis(ap=idx_i[:, k:k + 1], axis=0),
                    )
                    if k == 0:
                        nc.scalar.activation(
                            out=acc[:], in_=vg[:], func=mybir.ActivationFunctionType.Copy,
                            scale=attn_t[:, 0:1],
                        )
                    else:
                        wt = wtmp_pool.tile([P, D], F32)
                        nc.scalar.activation(
                            out=wt[:], in_=vg[:], func=mybir.ActivationFunctionType.Copy,
                            scale=attn_t[:, k:k + 1],
                        )
                        nc.vector.tensor_tensor(
                            out=acc[:], in0=acc[:], in1=wt[:], op=mybir.AluOpType.add
                        )

                qblk2 = big2_pool.tile([P, D], F32, tag="qblk2")
                nc.sync.dma_start(out=qblk2[:], in_=q_ext[b * P:(b + 1) * P, :])
                nc.vector.tensor_tensor(out=acc[:], in0=acc[:], in1=qblk2[:], op=mybir.AluOpType.add)

                s1 = small_pool.tile([P, 1], F32, tag="s1")
                nc.vector.tensor_reduce(
                    out=s1[:], in_=acc[:], axis=mybir.AxisListType.XYZW,
                    op=mybir.AluOpType.add,
                )
                negmu = small_pool.tile([P, 1], F32, tag="negmu")
                nc.vector.tensor_scalar(
                    out=negmu[:], in0=s1[:], scalar1=-1.0 / D, scalar2=None,
                    op0=mybir.AluOpType.mult,
                )
                xc = big2_pool.tile([P, D], F32, tag="xc")
                nc.vector.tensor_scalar(
                    out=xc[:], in0=acc[:], scalar1=negmu[:, 0:1], scalar2=None,
                    op0=mybir.AluOpType.add,
                )
                junk2 = junk_pool.tile([P, D], BF16)
                ssqv = small_pool.tile([P, 1], F32, tag="ssqv")
                nc.scalar.activation(
                    out=junk2[:], in_=xc[:],
                    func=mybir.ActivationFunctionType.Square, accum_out=ssqv[:],
                )
                sdv = small_pool.tile([P, 1], F32, tag="sdv")
                nc.scalar.activation(
                    out=sdv[:], in_=ssqv[:], func=mybir.ActivationFunctionType.Sqrt,
                    scale=1.0 / D, bias=eps_t[:, 0:1],
                )
                rsd = small_pool.tile([P, 1], F32, tag="rsd")
                nc.vector.reciprocal(out=rsd[:], in_=sdv[:])
                nc.vector.tensor_scalar(
                    out=xc[:], in0=xc[:], scalar1=rsd[:, 0:1], scalar2=None,
                    op0=mybir.AluOpType.mult,
                )
                nc.vector.tensor_tensor(out=xc[:], in0=xc[:], in1=gamma_bc[:], op=mybir.AluOpType.mult)
                nc.vector.tensor_tensor(out=xc[:], in0=xc[:], in1=beta_bc[:], op=mybir.AluOpType.add)
                nc.sync.dma_start(out=out_ext[b * P:(b + 1) * P, :], in_=xc[:])

    nc.finalize()
    return nc


def _install_ntff_shim():
    """Provide antenv.axon_hooks (NTFF profiling hook) if the image lacks it."""
    import contextlib
    import ctypes
    import sys as _sys
    import types

    try:
        from antenv.axon_hooks import get_axon_ntff_profile_hook  # noqa: F401
        return
    except ImportError:
        pass

    so_path = "/opt/axon/libaxon_pjrt.so"
    lib = ctypes.CDLL(so_path)
    if not hasattr(lib, "axon_start_nrt_profile"):
        return
    lib.axon_start_nrt_profile.argtypes = [
        ctypes.POINTER(ctypes.c_int64), ctypes.c_size_t,
    ]
    lib.axon_start_nrt_profile.restype = ctypes.c_int64
    lib.axon_stop_nrt_profile.argtypes = [ctypes.c_char_p]
    lib.axon_stop_nrt_profile.restype = ctypes.c_int64

    @contextlib.contextmanager
    def _hook(output_dir, device_ids):
        import jax
        jax.devices()
        if device_ids:
            ids = (ctypes.c_int64 * len(device_ids))(*device_ids)
            rc = lib.axon_start_nrt_profile(ids, len(device_ids))
        else:
            rc = lib.axon_start_nrt_profile(None, 0)
        if rc != 0:
            raise RuntimeError(f"axon_start_nrt_profile rc={rc}")
        try:
            yield
        finally:
            n = lib.axon_stop_nrt_profile(str(output_dir).encode())
            print(f"profile: {n} file(s) written to {output_dir}", file=sys.stderr)

    import antenv
    mod = types.ModuleType("antenv.axon_hooks")
    mod._HOOK = _hook
    mod.get_axon_ntff_profile_hook = lambda: mod._HOOK
    mod.set_axon_ntff_profile_hook = lambda h: setattr(mod, "_HOOK", h)
    _sys.modules["antenv.axon_hooks"] = mod
    antenv.axon_hooks = mod



def _get_nc():
    if "nc" not in _NC_CACHE:
        _NC_CACHE["nc"] = build_nc()
    return _NC_CACHE["nc"]


def kernel(query, memory_keys, memory_values, temperature, ln_gamma, ln_beta,
           trace=False):
    global LAST_RESULT
    query = np.ascontiguousarray(np.asarray(query, dtype=np.float32))
    keys = np.ascontiguousarray(np.asarray(memory_keys, dtype=np.float32))
    vals = np.ascontiguousarray(np.asarray(memory_values, dtype=np.float32))
    temp = np.asarray(temperature, dtype=np.float32).reshape(1, 1)
    gamma = np.asarray(ln_gamma, dtype=np.float32).reshape(1, D)
    beta = np.asarray(ln_beta, dtype=np.float32).reshape(1, D)

    if trace:
        try:
            _install_ntff_shim()
        except Exception as e:
            print(f"ntff shim failed ({e}); running without trace", file=sys.stderr)
            trace = False

    nc = _get_nc()
    in_maps = []
    for i in range(N_CORES):
        in_maps.append({
            "query": query[i * N_PER_CORE:(i + 1) * N_PER_CORE],
            "memory_keys": keys,
            "memory_values": vals,
            "temperature": temp,
            "ln_gamma": gamma,
            "ln_beta": beta,
        })
    res = run_bass_kernel_spmd(nc, in_maps, core_ids=list(range(N_CORES)), trace=trace)
    LAST_RESULT = res
    out = np.concatenate([res.results[i]["out"] for i in range(N_CORES)], axis=0)
    attn = np.concatenate([res.results[i]["attn"] for i in range(N_CORES)], axis=0)
    return out, attn
